# revision 40
# baseline (speedup 1.0000x reference)
"""Trainium2 Bass kernel for nn_MultiHeadEncDecAttention.

Problem (full shapes):
  x:[4,512,8,256] z:[256,512,32] w_q_w:[256,256] fc_w:[256,256] (+biases, LN params)
  q = x@w_q_w.T (+b) -> [h,v,b,s,dq]; attn = softmax(q@z^T/sqrt(dq)); out = attn@z
  o2 = concat_h(out)@fc_w.T (+b); y = LN(o2 + x)*gamma + beta

Sharding: split on n_verts (nv=8) across the 8 cores - every stage
(q-proj, attention, fc, LN) is independent per vert, so zero cross-core comms.

v2 design (per core, r = b*512+s in [0,2048)):
  Heads processed in order h_perm=[0,4,1,5,2,6,3,7]; head at order index o
  lives in tile T=o//4 at partition band 32*(o%4). Consecutive pairs hit
  different PE row-groups -> row-tiled K=32 logits matmuls overlap.
  Logits emitted per (pair, s-half 256) unit = [512t, 256s] in one 2-bank
  PSUM tile; exp on ACT per unit.
  AV: pairs (h=k, h=k+4) col-tiled at out partition bases {0,64} into one
  PSUM bank ([33] rows each + ones-row sums at rows 32/96); one DVE cast
  [97,512] lands both heads directly in outcT tile k (no shift DMAs).
  fc: 4 accumulating K=128 matmuls over the 4 outcT tiles (fcT zero-padded
  on the junk rows); norm multiply zeroes junk rows via indicator-matmul B.
"""

import sys

sys.path.insert(0, "/opt/trn_rl_repo")

from contextlib import ExitStack

import ml_dtypes
import numpy as np

import concourse.bass as bass
import concourse.tile as tile
from concourse import mybir

F32 = mybir.dt.float32
BF16 = mybir.dt.bfloat16
AX = mybir.AluOpType
AF = mybir.ActivationFunctionType

N_HEAD = 8
D_Q = 32
D_IN = 256
BS = 4
SEG = 512
NV = 8
LN_EPS = 1e-5
R = BS * SEG  # 2048 rows per core
NCORES = 8
INV_TEMP = 1.0 / np.sqrt(np.float32(D_Q))
H_PERM = [0, 4, 1, 5, 2, 6, 3, 7]  # head for order index o

_prog_cache = {}


def _build(use_wqb: bool, use_gb: bool):
    from concourse import bacc

    nc = bacc.Bacc("TRN2", target_bir_lowering=False, debug=False)

    d_xT = nc.dram_tensor("xT", [2, 128, R], BF16, kind="ExternalInput").ap()
    d_xres = nc.dram_tensor("xres", [128, 16 * 256], F32, kind="ExternalInput").ap()
    d_zT = nc.dram_tensor("zT", [2, 128, 2048], BF16, kind="ExternalInput").ap()
    d_zA = nc.dram_tensor("zA", [4, 128, 66 * 16], BF16, kind="ExternalInput").ap()
    d_wqT = nc.dram_tensor("wqT", [2, 128, 256], BF16, kind="ExternalInput").ap()
    d_wqb = nc.dram_tensor("wqb", [128, 2], F32, kind="ExternalInput").ap()
    d_fcT = nc.dram_tensor("fcT", [4, 128, D_IN], BF16, kind="ExternalInput").ap()
    d_gbb = nc.dram_tensor("gbb", [128, 512], F32, kind="ExternalInput").ap()
    d_ind8 = nc.dram_tensor("ind8", [8, 512], BF16, kind="ExternalInput").ap()
    d_y = nc.dram_tensor("y", [R, D_IN], F32, kind="ExternalOutput").ap()

    with tile.TileContext(nc) as tc, ExitStack() as ctx:
        P = ctx.enter_context  # noqa

        big = P(tc.tile_pool(name="big", bufs=1))
        lgp = P(tc.tile_pool(name="lgp", bufs=3, space="PSUM"))
        avp = P(tc.tile_pool(name="avp", bufs=2, space="PSUM"))
        expp = P(tc.tile_pool(name="expp", bufs=6))
        smp = P(tc.tile_pool(name="smp", bufs=2))
        stp = P(tc.tile_pool(name="stp", bufs=2))
        outp = P(tc.tile_pool(name="outp", bufs=3))

        # ---- persistent SBUF tiles + input DMAs
        eps_t = big.tile([128, 1], F32)
        nc.vector.memset(eps_t[:], float(LN_EPS))
        dummy_t = big.tile([128, 1], F32)
        # early Exp so the ACT table set loads during the DMA phase
        nc.scalar.activation(dummy_t[:], eps_t[:], AF.Exp)

        # tile for the PE warm-up burst
        warm_t = big.tile([128, 512], BF16, name="warm")
        nc.gpsimd.memset(warm_t[:], 0.0)

        wqT_t = [big.tile([128, 256], BF16, name=f"wqT{k}") for k in range(2)]
        for k in range(2):
            nc.sync.dma_start(wqT_t[k][:], d_wqT[k])
        # xT as separate per-chunk tiles so qproj chunk n depends only on its
        # own DMA (dep tracking for DMA writes is whole-tile)
        xT_t = [
            [big.tile([128, 512], BF16, name=f"xT{k}_{n}") for n in range(4)]
            for k in range(2)
        ]
        for k in range(2):
            nc.gpsimd.dma_start(xT_t[k][0][:], d_xT[k, :, 0:512])
        wqb_t = big.tile([128, 2], F32)
        if use_wqb:
            nc.sync.dma_start(wqb_t[:], d_wqb)
        # remaining inputs split across the sync and gpsimd issue queues
        zT_t = [big.tile([128, 2048], BF16, name=f"zT{u}") for u in range(2)]
        nc.gpsimd.dma_start(zT_t[0][:], d_zT[0])
        for n in range(1, 4):
            for k in range(2):
                nc.sync.dma_start(xT_t[k][n][:], d_xT[k, :, 512 * n : 512 * (n + 1)])
        zA_t = [big.tile([128, 66 * 16], BF16, name=f"zA{c}") for c in range(4)]
        for c in range(4):
            nc.gpsimd.dma_start(zA_t[c][:], d_zA[c])
        nc.gpsimd.dma_start(zT_t[1][:], d_zT[1])

        ind8_t = big.tile([8, 512], BF16)
        nc.sync.dma_start(ind8_t[:], d_ind8)
        fcT_t = [big.tile([128, D_IN], BF16, name=f"fcT{e}") for e in range(4)]
        for e in range(4):
            nc.sync.dma_start(fcT_t[e][:], d_fcT[e])
        gbb_t = big.tile([128, 512], F32)
        if use_gb:
            nc.sync.dma_start(gbb_t[:], d_gbb)
        xres_t = big.tile([128, 16 * 256], F32)
        nc.gpsimd.dma_start(xres_t[:], d_xres)

        qT_t = [big.tile([128, R], BF16, name=f"qT{u}") for u in range(2)]
        # outcT tile k: head k rows 0:32, sums row 32; head k+4 rows 64:96,
        # sums row 96; rows 33:63 / 97:127 junk (zeroed by the norm multiply)
        outcT = [big.tile([128, R], BF16, name=f"outcT{e}") for e in range(4)]
        yhold = big.tile([128, 16 * 256], F32)
        mvall = big.tile([128, 32], F32)

        def mm(out, lhsT, rhs, **kw):
            nc.tensor.matmul(out, lhsT, rhs, skip_group_check=True, **kw)

        # zero the never-matmul-written rows of the two av PSUM slots once,
        # so the full-partition cast below never reads non-finite stale PSUM
        for _ in range(2):
            av0 = avp.tile([128, 512], F32, tag="avb", name="av_init")
            nc.vector.memset(av0[32:64, :], 0.0)
            nc.vector.memset(av0[96:128, :], 0.0)


        # PE warm-up burst: ~10 dummy matmuls on junk data with no input deps.
        # They run during the preamble/DMA phase and push the HAM activity
        # window past its busy threshold so real matmuls start at 2.4 GHz.
        for w in range(10):
            wp = lgp.tile([128, 512], F32, tag="lg", name="warmmm")
            mm(wp[:], warm_t[:, 0:128], warm_t[:], start=True, stop=True)

        # ---- q projection chunk: qT[tile T][:, 512n:512n+512]
        def emit_qproj(T, n):
            qp = avp.tile([128, 512], F32, tag="avb", name="qp")
            for k in range(2):
                mm(
                    qp[:],
                    wqT_t[k][:, 128 * T : 128 * (T + 1)],
                    xT_t[k][n][:],
                    start=(k == 0),
                    stop=(k == 1),
                )
            dst = qT_t[T][:, 512 * n : 512 * (n + 1)]
            if use_wqb:
                nc.vector.tensor_scalar(
                    dst, qp[:], wqb_t[:, T : T + 1], 0.0, AX.add, AX.add
                )
            else:
                nc.vector.tensor_copy(dst, qp[:])

        # ---- logits + exp for one (order-index o, batch b, s-half sh) unit
        expt_tiles = {}

        def emit_logits_exp_unit(b, o, th):
            # unit = t-half: logits[t-chunks 2th:2th+2, all 512 s] for pair o
            T, beta = o // 4, 32 * (o % 4)
            if th == 0:
                expt_tiles[(b, o)] = expp.tile([128, 2048], BF16, name="expt")
            expt = expt_tiles[(b, o)]
            lt = lgp.tile([128, 1024], F32, tag="lg", name="lt")
            for j in range(2):
                c = 2 * th + j
                mm(
                    lt[:, 512 * j : 512 * (j + 1)],
                    zT_t[T][beta : beta + 32, 512 * b + 128 * c : 512 * b + 128 * (c + 1)],
                    qT_t[T][beta : beta + 32, 512 * b : 512 * (b + 1)],
                    start=True,
                    stop=True,
                    tile_position=(beta, 0),
                )
            nc.scalar.activation(
                expt[:, 1024 * th : 1024 * (th + 1)],
                lt[:],
                AF.Exp,
                scale=float(INV_TEMP),
            )

        # ---- AV for av-pair g of batch b (heads g and g+4, col-tiled {0,64})
        # split into two emission halves so the matmuls interleave between
        # later logits units (AV of group G-2 never waits on anything)
        av_state = {}

        def emit_av_half(G, half):
            b, g = divmod(G, 4)
            q = 4 * b + g
            if half == 0:
                av_state[G] = (
                    avp.tile([128, 512], F32, tag="avb", name="av"),
                    expt_tiles.pop((b, 2 * g)),
                    expt_tiles.pop((b, 2 * g + 1)),
                )
            av, eA, eB = av_state[G]
            for c in (2 * half, 2 * half + 1):
                mm(
                    av[0:33, :],
                    zA_t[c][:, 66 * q : 66 * q + 33],
                    eA[:, 512 * c : 512 * (c + 1)],
                    start=(c == 0),
                    stop=(c == 3),
                )
                mm(
                    av[64:97, :],
                    zA_t[c][:, 66 * q + 33 : 66 * q + 66],
                    eB[:, 512 * c : 512 * (c + 1)],
                    start=(c == 0),
                    stop=(c == 3),
                )
            if half == 1:
                nc.vector.tensor_copy(outcT[g][:, 512 * b : 512 * (b + 1)], av[:])
                del av_state[G]

        # ---- epilogue pieces for batch b
        sums_tiles = {}

        def emit_sums_dma(b):
            sums_b = smp.tile([8, 512], BF16, tag="sums", name="sums_b")
            sums_tiles[b] = sums_b
            for k in range(4):
                src = outcT[k][32:97:64, 512 * b : 512 * (b + 1)]
                eng = nc.sync if k % 2 == 0 else nc.gpsimd
                eng.dma_start(sums_b[2 * k : 2 * k + 2, :], src)

        B_tiles = {}

        def emit_norm_recip(b):
            sums_b = sums_tiles.pop(b)
            sumf = smp.tile([8, 512], F32, name="sumf")
            nc.vector.tensor_copy(sumf[:], sums_b[:])
            recf = smp.tile([8, 512], F32, name="recf")
            nc.vector.reciprocal_approx_fast(recf[:], sumf[:])
            recb = smp.tile([8, 512], BF16, name="recb")
            nc.vector.tensor_copy(recb[:], recf[:])
            B_tiles[b] = recb

        def emit_norm_mult(b, k):
            recb = B_tiles[b]
            Bt = avp.tile([128, 512], F32, tag="avb", name="Bt")
            mm(Bt[:], ind8_t[:, 128 * k : 128 * (k + 1)], recb[:], start=True, stop=True)
            sl = outcT[k][:, 512 * b : 512 * (b + 1)]
            nc.vector.tensor_tensor(sl, sl, Bt[:], AX.mult)

        def emit_fc_chunk(b, sc):
            ci = 4 * b + sc
            reg = avp.tile([128, 512], F32, tag="avb", name="fcp")[:, 0:256]
            for k in range(4):
                mm(
                    reg[:],
                    outcT[k][:, 512 * b + 128 * sc : 512 * b + 128 * (sc + 1)],
                    fcT_t[k][:],
                    start=(k == 0),
                    stop=(k == 3),
                )
            ysl = yhold[:, 256 * ci : 256 * (ci + 1)]
            nc.vector.tensor_tensor(
                ysl, reg[:], xres_t[:, 256 * ci : 256 * (ci + 1)], AX.add
            )
            st6 = stp.tile([128, 6], F32, name="st6")
            nc.vector.bn_stats(st6[:], ysl)
            nc.vector.bn_aggr(mvall[:, 2 * ci : 2 * ci + 2], st6[:])

        # per-chunk LN for the tail: rstd for one 128-row chunk only, so the
        # chunk's output DMA can issue while the next chunk's fc is on the PE
        def emit_ln_chunk(b, sc, eng):
            ci = 4 * b + sc
            va = stp.tile([128, 1], F32, tag="va1", name="va1")
            nc.vector.tensor_scalar(
                va[:], mvall[:, 2 * ci + 1 : 2 * ci + 2], eps_t[:], 0.0, AX.add, AX.add
            )
            y = stp.tile([128, 1], F32, tag="ny1", name="ny1")
            nc.vector.reciprocal_approx_fast(y[:], va[:])
            for _ in range(3):
                t1 = stp.tile([128, 1], F32, tag="nt11", name="nt11")
                nc.vector.tensor_tensor(t1[:], y[:], y[:], AX.mult)
                nc.vector.tensor_tensor(t1[:], t1[:], va[:], AX.mult)
                nc.vector.tensor_scalar(t1[:], t1[:], -0.5, 1.5, AX.mult, AX.add)
                nc.vector.tensor_tensor(y[:], y[:], t1[:], AX.mult)
            ysl = yhold[:, 256 * ci : 256 * (ci + 1)]
            yo = outp.tile([128, 256], F32, name="yo")
            nc.vector.tensor_scalar(
                yo[:], ysl, mvall[:, 2 * ci : 2 * ci + 1], y[:, 0:1],
                AX.subtract, AX.mult,
            )
            eng.dma_start(d_y[128 * ci : 128 * (ci + 1), :], yo[:])

        def emit_ln_finish(b):
            # rstd = 1/sqrt(var+eps) on DVE (recip-approx seed + 2 Newton
            # rsqrt iterations) - avoids the ACT sqrt table switch entirely.
            mvb = mvall[:, 8 * b : 8 * (b + 1)].rearrange("p (c two) -> p c two", two=2)
            va = stp.tile([128, 4], F32, tag="va", name="va")
            nc.vector.tensor_scalar(va[:], mvb[:, :, 1:2], eps_t[:], 0.0, AX.add, AX.add)
            y = stp.tile([128, 4], F32, tag="ny", name="ny")
            nc.vector.reciprocal_approx_fast(y[:], va[:])
            for _ in range(3):
                t1 = stp.tile([128, 4], F32, tag="nt1", name="nt1")
                nc.vector.tensor_tensor(t1[:], y[:], y[:], AX.mult)
                nc.vector.tensor_tensor(t1[:], t1[:], va[:], AX.mult)
                nc.vector.tensor_scalar(t1[:], t1[:], -0.5, 1.5, AX.mult, AX.add)
                nc.vector.tensor_tensor(y[:], y[:], t1[:], AX.mult)
            for sc in range(4):
                ci = 4 * b + sc
                ysl = yhold[:, 256 * ci : 256 * (ci + 1)]
                yo = outp.tile([128, 256], F32, name="yo")
                if use_gb:
                    t2 = outp.tile([128, 256], F32, tag="t1", name="t2")
                    nc.vector.scalar_tensor_tensor(
                        t2[:], ysl, mvall[:, 2 * ci : 2 * ci + 1], gbb_t[:, 0:256],
                        AX.subtract, AX.mult,
                    )
                    nc.vector.scalar_tensor_tensor(
                        yo[:], t2[:], y[:, sc : sc + 1], gbb_t[:, 256:512],
                        AX.mult, AX.add,
                    )
                else:
                    nc.vector.tensor_scalar(
                        yo[:], ysl, mvall[:, 2 * ci : 2 * ci + 1], y[:, sc : sc + 1],
                        AX.subtract, AX.mult,
                    )
                eng = nc.gpsimd if sc % 2 == 0 else nc.sync
                eng.dma_start(d_y[128 * ci : 128 * (ci + 1), :], yo[:])

        # ---- main schedule -------------------------------------------------
        # qproj for batch 0 first (only needs wqT + xT chunk 0)
        emit_qproj(0, 0)
        emit_qproj(1, 0)

        def emit_av(b, g):
            emit_av_half(4 * b + g, 0)
            emit_av_half(4 * b + g, 1)

        # epilogue pieces of batch b-1, spread across batch b's groups
        def emit_epilogue_piece(b, g):
            if b < 0:
                return
            if g == 0:
                # DVE-only: the recip chain runs while PE streams the next
                # group, so the B matmuls at g=1 never stall the PE queue
                emit_norm_recip(b)
            elif g == 1:
                for k in range(4):
                    emit_norm_mult(b, k)
                emit_fc_chunk(b, 0)
            elif g == 2:
                emit_fc_chunk(b, 1)
                emit_fc_chunk(b, 2)
            else:
                emit_fc_chunk(b, 3)
                emit_ln_finish(b)

        for b in range(BS):
            for g in range(4):
                emit_logits_exp_unit(b, 2 * g, 0)
                emit_logits_exp_unit(b, 2 * g + 1, 0)
                if b < BS - 1 and g == 1:
                    emit_qproj(0, b + 1)
                emit_logits_exp_unit(b, 2 * g, 1)
                emit_logits_exp_unit(b, 2 * g + 1, 1)
                if b < BS - 1 and g == 2:
                    emit_qproj(1, b + 1)
                # AV of the previous group (pipelined one group back)
                if g > 0:
                    emit_av(b, g - 1)
                elif b > 0:
                    emit_av(b - 1, 3)
                    emit_sums_dma(b - 1)
                emit_epilogue_piece(b - 1, g)
        emit_av(BS - 1, 3)
        emit_sums_dma(BS - 1)
        if use_gb:
            for g in range(4):
                emit_epilogue_piece(BS - 1, g)
        else:
            # low-latency tail: fc and per-chunk LN interleaved, output DMAs
            # spread over three issue queues (ACT is idle here)
            emit_norm_recip(BS - 1)
            for k in range(4):
                emit_norm_mult(BS - 1, k)
            dma_engs = [nc.gpsimd, nc.sync, nc.scalar, nc.sync]
            for sc in range(4):
                emit_fc_chunk(BS - 1, sc)
                emit_ln_chunk(BS - 1, sc, dma_engs[sc])

    nc.compile()
    return nc


def _prep_core(x, z, fc_b, v):
    """Build the per-core input map (host-side layout packing) for vert v."""
    bf = ml_dtypes.bfloat16
    xv = np.ascontiguousarray(x[:, :, v, :]).reshape(R, D_IN)  # [r, d]
    xT = np.ascontiguousarray(xv.T).astype(bf).reshape(2, 128, R)  # [d, r]
    xres = np.ascontiguousarray(
        (xv + fc_b[None, :]).reshape(16, 128, 256).transpose(1, 0, 2).reshape(128, 16 * 256)
    )
    zv = z.reshape(N_HEAD, NV, BS, SEG, D_Q)[:, v]  # [h, b, t, d]
    zTp = zv.transpose(0, 1, 3, 2)  # [h, b, d, t]
    zT = np.zeros((2, 4, 32, 4, 512), bf)
    for o, h in enumerate(H_PERM):
        for b in range(BS):
            zT[o // 4, o % 4, :, b] = zTp[h, b]
    zT = np.ascontiguousarray(zT.reshape(2, 128, 2048))
    # zA: per av-pair slot q = 4b+g: [headA(g) 33 | headB(g+4) 33]
    zA = np.zeros((4, 128, 66 * 16), bf)
    za_full = np.concatenate(
        [zv, np.ones((N_HEAD, BS, SEG, 1), np.float32)], axis=-1
    ).astype(bf)  # [h, b, t, 33]
    for b in range(BS):
        for g in range(4):
            q = 4 * b + g
            for c in range(4):
                zA[c, :, 66 * q : 66 * q + 33] = za_full[g, b, 128 * c : 128 * (c + 1), :]
                zA[c, :, 66 * q + 33 : 66 * q + 66] = za_full[
                    g + 4, b, 128 * c : 128 * (c + 1), :
                ]
    return {"xT": xT, "xres": xres, "zT": zT, "zA": zA}


def kernel(x, z, w_q_w, w_q_b, fc_w, fc_b, ln_gamma, ln_beta, _trace=False, _tmpdir=None):
    from concourse.bass_utils import run_bass_kernel_spmd

    x = np.asarray(x, np.float32)
    z = np.asarray(z, np.float32)
    w_q_w = np.asarray(w_q_w, np.float32)
    w_q_b = np.asarray(w_q_b, np.float32)
    fc_w = np.asarray(fc_w, np.float32)
    fc_b = np.asarray(fc_b, np.float32)
    ln_gamma = np.asarray(ln_gamma, np.float32)
    ln_beta = np.asarray(ln_beta, np.float32)

    use_wqb = bool(np.any(w_q_b != 0.0))
    use_gb = bool(np.any(ln_gamma != 1.0) or np.any(ln_beta != 0.0))

    key = (use_wqb, use_gb)
    if key not in _prog_cache:
        _prog_cache[key] = _build(use_wqb, use_gb)
    nc = _prog_cache[key]

    bf = ml_dtypes.bfloat16
    # e' permutation: tile T col j -> head H_PERM[4T + j//32], dq j%32
    eperm = np.zeros(256, np.int64)
    for T in range(2):
        for j in range(128):
            o = 4 * T + j // 32
            eperm[128 * T + j] = 32 * H_PERM[o] + j % 32
    wqT = np.ascontiguousarray(w_q_w.T[:, eperm]).astype(bf).reshape(2, 128, 256)
    wqb_p = np.zeros((128, 2), np.float32)
    for T in range(2):
        wqb_p[:, T] = w_q_b[eperm[128 * T : 128 * (T + 1)]]
    # fcT tile k: rows 0:32 = fc rows of head k, rows 64:96 = head k+4, else 0
    fcT_full = fc_w.T  # [e, d_in]
    fcT = np.zeros((4, 128, D_IN), np.float32)
    for k in range(4):
        fcT[k, 0:32] = fcT_full[32 * k : 32 * (k + 1)]
        fcT[k, 64:96] = fcT_full[32 * (k + 4) : 32 * (k + 5)]
    fcT = fcT.astype(bf)
    # ind8 tile: B_k = ind8[:, 128k:128k+128].T @ recb
    ind8 = np.zeros((8, 512), bf)
    for k in range(4):
        ind8[2 * k, 128 * k : 128 * k + 32] = 1.0
        ind8[2 * k + 1, 128 * k + 64 : 128 * k + 96] = 1.0
    shared = {
        "wqT": wqT,
        "wqb": wqb_p,
        "ind8": ind8,
        "fcT": fcT,
        "gbb": np.ascontiguousarray(
            np.concatenate(
                [
                    np.broadcast_to(ln_gamma, (128, 256)),
                    np.broadcast_to(ln_beta, (128, 256)),
                ],
                axis=1,
            )
        ),
    }
    in_maps = []
    for v in range(NCORES):
        m = dict(shared)
        m.update(_prep_core(x, z, fc_b, v))
        in_maps.append(m)

    res = run_bass_kernel_spmd(
        nc,
        in_maps,
        core_ids=list(range(NCORES)),
        trace=_trace,
        tmpdir=_tmpdir,
    )
    out = np.empty((BS, SEG, NV, D_IN), np.float32)
    for v in range(NCORES):
        out[:, :, v, :] = res.results[v]["y"].reshape(BS, SEG, D_IN)
    kernel._last_result = res
    return out


# revision 41
# speedup vs baseline: 1.0053x; 1.0053x over previous
"""Trainium2 Bass kernel for nn_MultiHeadEncDecAttention.

Problem (full shapes):
  x:[4,512,8,256] z:[256,512,32] w_q_w:[256,256] fc_w:[256,256] (+biases, LN params)
  q = x@w_q_w.T (+b) -> [h,v,b,s,dq]; attn = softmax(q@z^T/sqrt(dq)); out = attn@z
  o2 = concat_h(out)@fc_w.T (+b); y = LN(o2 + x)*gamma + beta

Sharding: split on n_verts (nv=8) across the 8 cores - every stage
(q-proj, attention, fc, LN) is independent per vert, so zero cross-core comms.

v2 design (per core, r = b*512+s in [0,2048)):
  Heads processed in order h_perm=[0,4,1,5,2,6,3,7]; head at order index o
  lives in tile T=o//4 at partition band 32*(o%4). Consecutive pairs hit
  different PE row-groups -> row-tiled K=32 logits matmuls overlap.
  Logits emitted per (pair, s-half 256) unit = [512t, 256s] in one 2-bank
  PSUM tile; exp on ACT per unit.
  AV: pairs (h=k, h=k+4) col-tiled at out partition bases {0,64} into one
  PSUM bank ([33] rows each + ones-row sums at rows 32/96); one DVE cast
  [97,512] lands both heads directly in outcT tile k (no shift DMAs).
  fc: 4 accumulating K=128 matmuls over the 4 outcT tiles (fcT zero-padded
  on the junk rows); norm multiply zeroes junk rows via indicator-matmul B.
"""

import sys

sys.path.insert(0, "/opt/trn_rl_repo")

from contextlib import ExitStack

import ml_dtypes
import numpy as np

import concourse.bass as bass
import concourse.tile as tile
from concourse import mybir

F32 = mybir.dt.float32
BF16 = mybir.dt.bfloat16
AX = mybir.AluOpType
AF = mybir.ActivationFunctionType

N_HEAD = 8
D_Q = 32
D_IN = 256
BS = 4
SEG = 512
NV = 8
LN_EPS = 1e-5
R = BS * SEG  # 2048 rows per core
NCORES = 8
INV_TEMP = 1.0 / np.sqrt(np.float32(D_Q))
H_PERM = [0, 4, 1, 5, 2, 6, 3, 7]  # head for order index o

_prog_cache = {}


def _build(use_wqb: bool, use_gb: bool):
    from concourse import bacc

    nc = bacc.Bacc("TRN2", target_bir_lowering=False, debug=False)

    d_xT = nc.dram_tensor("xT", [2, 128, R], BF16, kind="ExternalInput").ap()
    d_xres = nc.dram_tensor("xres", [128, 16 * 256], F32, kind="ExternalInput").ap()
    d_zT = nc.dram_tensor("zT", [2, 128, 2048], BF16, kind="ExternalInput").ap()
    d_zA = nc.dram_tensor("zA", [4, 128, 66 * 16], BF16, kind="ExternalInput").ap()
    d_wqT = nc.dram_tensor("wqT", [2, 128, 256], BF16, kind="ExternalInput").ap()
    d_wqb = nc.dram_tensor("wqb", [128, 2], F32, kind="ExternalInput").ap()
    d_fcT = nc.dram_tensor("fcT", [4, 128, D_IN], BF16, kind="ExternalInput").ap()
    d_gbb = nc.dram_tensor("gbb", [128, 512], F32, kind="ExternalInput").ap()
    d_ind8 = nc.dram_tensor("ind8", [8, 512], BF16, kind="ExternalInput").ap()
    d_y = nc.dram_tensor("y", [R, D_IN], F32, kind="ExternalOutput").ap()

    with tile.TileContext(nc) as tc, ExitStack() as ctx:
        P = ctx.enter_context  # noqa

        big = P(tc.tile_pool(name="big", bufs=1))
        lgp = P(tc.tile_pool(name="lgp", bufs=3, space="PSUM"))
        avp = P(tc.tile_pool(name="avp", bufs=2, space="PSUM"))
        expp = P(tc.tile_pool(name="expp", bufs=6))
        smp = P(tc.tile_pool(name="smp", bufs=2))
        stp = P(tc.tile_pool(name="stp", bufs=2))
        outp = P(tc.tile_pool(name="outp", bufs=3))

        # ---- persistent SBUF tiles + input DMAs
        eps_t = big.tile([128, 1], F32)
        nc.vector.memset(eps_t[:], float(LN_EPS))
        dummy_t = big.tile([128, 1], F32)
        # early Exp so the ACT table set loads during the DMA phase
        nc.scalar.activation(dummy_t[:], eps_t[:], AF.Exp)

        # tile for the PE warm-up burst
        warm_t = big.tile([128, 512], BF16, name="warm")
        nc.gpsimd.memset(warm_t[:], 0.0)

        wqT_t = [big.tile([128, 256], BF16, name=f"wqT{k}") for k in range(2)]
        for k in range(2):
            nc.sync.dma_start(wqT_t[k][:], d_wqT[k])
        # xT as separate per-chunk tiles so qproj chunk n depends only on its
        # own DMA (dep tracking for DMA writes is whole-tile)
        xT_t = [
            [big.tile([128, 512], BF16, name=f"xT{k}_{n}") for n in range(4)]
            for k in range(2)
        ]
        for k in range(2):
            nc.gpsimd.dma_start(xT_t[k][0][:], d_xT[k, :, 0:512])
        wqb_t = big.tile([128, 2], F32)
        if use_wqb:
            nc.sync.dma_start(wqb_t[:], d_wqb)
        # remaining inputs split across the sync and gpsimd issue queues
        zT_t = [big.tile([128, 2048], BF16, name=f"zT{u}") for u in range(2)]
        nc.gpsimd.dma_start(zT_t[0][:], d_zT[0])
        for n in range(1, 4):
            for k in range(2):
                nc.sync.dma_start(xT_t[k][n][:], d_xT[k, :, 512 * n : 512 * (n + 1)])
        zA_t = [big.tile([128, 66 * 16], BF16, name=f"zA{c}") for c in range(4)]
        for c in range(4):
            nc.gpsimd.dma_start(zA_t[c][:], d_zA[c])
        nc.gpsimd.dma_start(zT_t[1][:], d_zT[1])

        ind8_t = big.tile([8, 512], BF16)
        nc.sync.dma_start(ind8_t[:], d_ind8)
        fcT_t = [big.tile([128, D_IN], BF16, name=f"fcT{e}") for e in range(4)]
        for e in range(4):
            nc.sync.dma_start(fcT_t[e][:], d_fcT[e])
        gbb_t = big.tile([128, 512], F32)
        if use_gb:
            nc.sync.dma_start(gbb_t[:], d_gbb)
        xres_t = big.tile([128, 16 * 256], F32)
        nc.gpsimd.dma_start(xres_t[:], d_xres)

        qT_t = [big.tile([128, R], BF16, name=f"qT{u}") for u in range(2)]
        # outcT tile k: head k rows 0:32, sums row 32; head k+4 rows 64:96,
        # sums row 96; rows 33:63 / 97:127 junk (zeroed by the norm multiply)
        outcT = [big.tile([128, R], BF16, name=f"outcT{e}") for e in range(4)]
        yhold = big.tile([128, 16 * 256], F32)
        mvall = big.tile([128, 32], F32)

        def mm(out, lhsT, rhs, **kw):
            nc.tensor.matmul(out, lhsT, rhs, skip_group_check=True, **kw)

        # zero the never-matmul-written rows of the two av PSUM slots once,
        # so the full-partition cast below never reads non-finite stale PSUM
        for _ in range(2):
            av0 = avp.tile([128, 512], F32, tag="avb", name="av_init")
            nc.vector.memset(av0[32:64, :], 0.0)
            nc.vector.memset(av0[96:128, :], 0.0)


        # PE warm-up burst: ~10 dummy matmuls on junk data with no input deps.
        # They run during the preamble/DMA phase and push the HAM activity
        # window past its busy threshold so real matmuls start at 2.4 GHz.
        for w in range(10):
            wp = lgp.tile([128, 512], F32, tag="lg", name="warmmm")
            mm(wp[:], warm_t[:, 0:128], warm_t[:], start=True, stop=True)

        # ---- q projection chunk: qT[tile T][:, 512n:512n+512]
        def emit_qproj(T, n):
            qp = avp.tile([128, 512], F32, tag="avb", name="qp")
            for k in range(2):
                mm(
                    qp[:],
                    wqT_t[k][:, 128 * T : 128 * (T + 1)],
                    xT_t[k][n][:],
                    start=(k == 0),
                    stop=(k == 1),
                )
            dst = qT_t[T][:, 512 * n : 512 * (n + 1)]
            if use_wqb:
                nc.vector.tensor_scalar(
                    dst, qp[:], wqb_t[:, T : T + 1], 0.0, AX.add, AX.add
                )
            else:
                nc.vector.tensor_copy(dst, qp[:])

        # ---- logits + exp for one (order-index o, batch b, s-half sh) unit
        expt_tiles = {}

        def emit_logits_exp_unit(b, o, th):
            # unit = t-half: logits[t-chunks 2th:2th+2, all 512 s] for pair o
            T, beta = o // 4, 32 * (o % 4)
            if th == 0:
                expt_tiles[(b, o)] = expp.tile([128, 2048], BF16, name="expt")
            expt = expt_tiles[(b, o)]
            lt = lgp.tile([128, 1024], F32, tag="lg", name="lt")
            for j in range(2):
                c = 2 * th + j
                mm(
                    lt[:, 512 * j : 512 * (j + 1)],
                    zT_t[T][beta : beta + 32, 512 * b + 128 * c : 512 * b + 128 * (c + 1)],
                    qT_t[T][beta : beta + 32, 512 * b : 512 * (b + 1)],
                    start=True,
                    stop=True,
                    tile_position=(beta, 0),
                )
            nc.scalar.activation(
                expt[:, 1024 * th : 1024 * (th + 1)],
                lt[:],
                AF.Exp,
                scale=float(INV_TEMP),
            )

        # ---- AV for av-pair g of batch b (heads g and g+4, col-tiled {0,64})
        # split into two emission halves so the matmuls interleave between
        # later logits units (AV of group G-2 never waits on anything)
        av_state = {}

        def emit_av_half(G, half):
            b, g = divmod(G, 4)
            q = 4 * b + g
            if half == 0:
                av_state[G] = (
                    avp.tile([128, 512], F32, tag="avb", name="av"),
                    expt_tiles.pop((b, 2 * g)),
                    expt_tiles.pop((b, 2 * g + 1)),
                )
            av, eA, eB = av_state[G]
            for c in (2 * half, 2 * half + 1):
                mm(
                    av[0:33, :],
                    zA_t[c][:, 66 * q : 66 * q + 33],
                    eA[:, 512 * c : 512 * (c + 1)],
                    start=(c == 0),
                    stop=(c == 3),
                )
                mm(
                    av[64:97, :],
                    zA_t[c][:, 66 * q + 33 : 66 * q + 66],
                    eB[:, 512 * c : 512 * (c + 1)],
                    start=(c == 0),
                    stop=(c == 3),
                )
            if half == 1:
                nc.vector.tensor_copy(outcT[g][:, 512 * b : 512 * (b + 1)], av[:])
                del av_state[G]

        # ---- epilogue pieces for batch b
        sums_tiles = {}

        def emit_sums_dma(b):
            sums_b = smp.tile([8, 512], BF16, tag="sums", name="sums_b")
            sums_tiles[b] = sums_b
            for k in range(4):
                src = outcT[k][32:97:64, 512 * b : 512 * (b + 1)]
                eng = nc.sync if k % 2 == 0 else nc.gpsimd
                eng.dma_start(sums_b[2 * k : 2 * k + 2, :], src)

        B_tiles = {}

        def emit_norm_recip(b):
            sums_b = sums_tiles.pop(b)
            sumf = smp.tile([8, 512], F32, name="sumf")
            nc.vector.tensor_copy(sumf[:], sums_b[:])
            recf = smp.tile([8, 512], F32, name="recf")
            nc.vector.reciprocal_approx_fast(recf[:], sumf[:])
            recb = smp.tile([8, 512], BF16, name="recb")
            nc.vector.tensor_copy(recb[:], recf[:])
            B_tiles[b] = recb

        def emit_norm_mult(b, k):
            recb = B_tiles[b]
            Bt = avp.tile([128, 512], F32, tag="avb", name="Bt")
            mm(Bt[:], ind8_t[:, 128 * k : 128 * (k + 1)], recb[:], start=True, stop=True)
            sl = outcT[k][:, 512 * b : 512 * (b + 1)]
            nc.vector.tensor_tensor(sl, sl, Bt[:], AX.mult)

        def emit_fc_chunk(b, sc):
            ci = 4 * b + sc
            reg = avp.tile([128, 512], F32, tag="avb", name="fcp")[:, 0:256]
            for k in range(4):
                mm(
                    reg[:],
                    outcT[k][:, 512 * b + 128 * sc : 512 * b + 128 * (sc + 1)],
                    fcT_t[k][:],
                    start=(k == 0),
                    stop=(k == 3),
                )
            ysl = yhold[:, 256 * ci : 256 * (ci + 1)]
            nc.vector.tensor_tensor(
                ysl, reg[:], xres_t[:, 256 * ci : 256 * (ci + 1)], AX.add
            )
            st6 = stp.tile([128, 6], F32, name="st6")
            nc.vector.bn_stats(st6[:], ysl)
            nc.vector.bn_aggr(mvall[:, 2 * ci : 2 * ci + 2], st6[:])

        # per-chunk LN for the tail: rstd for one 128-row chunk only, so the
        # chunk's output DMA can issue while the next chunk's fc is on the PE
        def emit_ln_chunk(b, sc, eng):
            ci = 4 * b + sc
            va = stp.tile([128, 1], F32, tag="va1", name="va1")
            nc.vector.tensor_scalar(
                va[:], mvall[:, 2 * ci + 1 : 2 * ci + 2], eps_t[:], 0.0, AX.add, AX.add
            )
            y = stp.tile([128, 1], F32, tag="ny1", name="ny1")
            nc.vector.reciprocal_approx_fast(y[:], va[:])
            for _ in range(3):
                t1 = stp.tile([128, 1], F32, tag="nt11", name="nt11")
                nc.vector.tensor_tensor(t1[:], y[:], y[:], AX.mult)
                nc.vector.tensor_tensor(t1[:], t1[:], va[:], AX.mult)
                nc.vector.tensor_scalar(t1[:], t1[:], -0.5, 1.5, AX.mult, AX.add)
                nc.vector.tensor_tensor(y[:], y[:], t1[:], AX.mult)
            ysl = yhold[:, 256 * ci : 256 * (ci + 1)]
            yo = outp.tile([128, 256], F32, name="yo")
            nc.vector.tensor_scalar(
                yo[:], ysl, mvall[:, 2 * ci : 2 * ci + 1], y[:, 0:1],
                AX.subtract, AX.mult,
            )
            eng.dma_start(d_y[128 * ci : 128 * (ci + 1), :], yo[:])

        def emit_ln_finish(b):
            # rstd = 1/sqrt(var+eps) on DVE (recip-approx seed + 2 Newton
            # rsqrt iterations) - avoids the ACT sqrt table switch entirely.
            mvb = mvall[:, 8 * b : 8 * (b + 1)].rearrange("p (c two) -> p c two", two=2)
            va = stp.tile([128, 4], F32, tag="va", name="va")
            nc.vector.tensor_scalar(va[:], mvb[:, :, 1:2], eps_t[:], 0.0, AX.add, AX.add)
            y = stp.tile([128, 4], F32, tag="ny", name="ny")
            nc.vector.reciprocal_approx_fast(y[:], va[:])
            for _ in range(3):
                t1 = stp.tile([128, 4], F32, tag="nt1", name="nt1")
                nc.vector.tensor_tensor(t1[:], y[:], y[:], AX.mult)
                nc.vector.tensor_tensor(t1[:], t1[:], va[:], AX.mult)
                nc.vector.tensor_scalar(t1[:], t1[:], -0.5, 1.5, AX.mult, AX.add)
                nc.vector.tensor_tensor(y[:], y[:], t1[:], AX.mult)
            for sc in range(4):
                ci = 4 * b + sc
                ysl = yhold[:, 256 * ci : 256 * (ci + 1)]
                yo = outp.tile([128, 256], F32, name="yo")
                if use_gb:
                    t2 = outp.tile([128, 256], F32, tag="t1", name="t2")
                    nc.vector.scalar_tensor_tensor(
                        t2[:], ysl, mvall[:, 2 * ci : 2 * ci + 1], gbb_t[:, 0:256],
                        AX.subtract, AX.mult,
                    )
                    nc.vector.scalar_tensor_tensor(
                        yo[:], t2[:], y[:, sc : sc + 1], gbb_t[:, 256:512],
                        AX.mult, AX.add,
                    )
                else:
                    nc.vector.tensor_scalar(
                        yo[:], ysl, mvall[:, 2 * ci : 2 * ci + 1], y[:, sc : sc + 1],
                        AX.subtract, AX.mult,
                    )
                eng = nc.gpsimd if sc % 2 == 0 else nc.sync
                eng.dma_start(d_y[128 * ci : 128 * (ci + 1), :], yo[:])

        # ---- main schedule -------------------------------------------------
        # qproj for batch 0 first (only needs wqT + xT chunk 0)
        emit_qproj(0, 0)
        emit_qproj(1, 0)

        def emit_av(b, g):
            emit_av_half(4 * b + g, 0)
            emit_av_half(4 * b + g, 1)

        # epilogue pieces of batch b-1, spread across batch b's groups
        def emit_epilogue_piece(b, g):
            if b < 0:
                return
            if g == 0:
                # DVE-only: the recip chain runs while PE streams the next
                # group, so the B matmuls at g=1 never stall the PE queue
                emit_norm_recip(b)
            elif g == 1:
                for k in range(4):
                    emit_norm_mult(b, k)
                emit_fc_chunk(b, 0)
            elif g == 2:
                emit_fc_chunk(b, 1)
                emit_fc_chunk(b, 2)
            else:
                emit_fc_chunk(b, 3)
                emit_ln_finish(b)

        for b in range(BS):
            for g in range(4):
                emit_logits_exp_unit(b, 2 * g, 0)
                emit_logits_exp_unit(b, 2 * g + 1, 0)
                if b < BS - 1 and g == 1:
                    emit_qproj(0, b + 1)
                emit_logits_exp_unit(b, 2 * g, 1)
                emit_logits_exp_unit(b, 2 * g + 1, 1)
                if b < BS - 1 and g == 2:
                    emit_qproj(1, b + 1)
                # AV of the previous group (pipelined one group back)
                if g > 0:
                    emit_av(b, g - 1)
                elif b > 0:
                    emit_av(b - 1, 3)
                    emit_sums_dma(b - 1)
                emit_epilogue_piece(b - 1, g)
        emit_av(BS - 1, 3)
        emit_sums_dma(BS - 1)
        for g in range(4):
            emit_epilogue_piece(BS - 1, g)

    nc.compile()
    return nc


def _prep_core(x, z, fc_b, v):
    """Build the per-core input map (host-side layout packing) for vert v."""
    bf = ml_dtypes.bfloat16
    xv = np.ascontiguousarray(x[:, :, v, :]).reshape(R, D_IN)  # [r, d]
    xT = np.ascontiguousarray(xv.T).astype(bf).reshape(2, 128, R)  # [d, r]
    xres = np.ascontiguousarray(
        (xv + fc_b[None, :]).reshape(16, 128, 256).transpose(1, 0, 2).reshape(128, 16 * 256)
    )
    zv = z.reshape(N_HEAD, NV, BS, SEG, D_Q)[:, v]  # [h, b, t, d]
    zTp = zv.transpose(0, 1, 3, 2)  # [h, b, d, t]
    zT = np.zeros((2, 4, 32, 4, 512), bf)
    for o, h in enumerate(H_PERM):
        for b in range(BS):
            zT[o // 4, o % 4, :, b] = zTp[h, b]
    zT = np.ascontiguousarray(zT.reshape(2, 128, 2048))
    # zA: per av-pair slot q = 4b+g: [headA(g) 33 | headB(g+4) 33]
    zA = np.zeros((4, 128, 66 * 16), bf)
    za_full = np.concatenate(
        [zv, np.ones((N_HEAD, BS, SEG, 1), np.float32)], axis=-1
    ).astype(bf)  # [h, b, t, 33]
    for b in range(BS):
        for g in range(4):
            q = 4 * b + g
            for c in range(4):
                zA[c, :, 66 * q : 66 * q + 33] = za_full[g, b, 128 * c : 128 * (c + 1), :]
                zA[c, :, 66 * q + 33 : 66 * q + 66] = za_full[
                    g + 4, b, 128 * c : 128 * (c + 1), :
                ]
    return {"xT": xT, "xres": xres, "zT": zT, "zA": zA}


def kernel(x, z, w_q_w, w_q_b, fc_w, fc_b, ln_gamma, ln_beta, _trace=False, _tmpdir=None):
    from concourse.bass_utils import run_bass_kernel_spmd

    x = np.asarray(x, np.float32)
    z = np.asarray(z, np.float32)
    w_q_w = np.asarray(w_q_w, np.float32)
    w_q_b = np.asarray(w_q_b, np.float32)
    fc_w = np.asarray(fc_w, np.float32)
    fc_b = np.asarray(fc_b, np.float32)
    ln_gamma = np.asarray(ln_gamma, np.float32)
    ln_beta = np.asarray(ln_beta, np.float32)

    use_wqb = bool(np.any(w_q_b != 0.0))
    use_gb = bool(np.any(ln_gamma != 1.0) or np.any(ln_beta != 0.0))

    key = (use_wqb, use_gb)
    if key not in _prog_cache:
        _prog_cache[key] = _build(use_wqb, use_gb)
    nc = _prog_cache[key]

    bf = ml_dtypes.bfloat16
    # e' permutation: tile T col j -> head H_PERM[4T + j//32], dq j%32
    eperm = np.zeros(256, np.int64)
    for T in range(2):
        for j in range(128):
            o = 4 * T + j // 32
            eperm[128 * T + j] = 32 * H_PERM[o] + j % 32
    wqT = np.ascontiguousarray(w_q_w.T[:, eperm]).astype(bf).reshape(2, 128, 256)
    wqb_p = np.zeros((128, 2), np.float32)
    for T in range(2):
        wqb_p[:, T] = w_q_b[eperm[128 * T : 128 * (T + 1)]]
    # fcT tile k: rows 0:32 = fc rows of head k, rows 64:96 = head k+4, else 0
    fcT_full = fc_w.T  # [e, d_in]
    fcT = np.zeros((4, 128, D_IN), np.float32)
    for k in range(4):
        fcT[k, 0:32] = fcT_full[32 * k : 32 * (k + 1)]
        fcT[k, 64:96] = fcT_full[32 * (k + 4) : 32 * (k + 5)]
    fcT = fcT.astype(bf)
    # ind8 tile: B_k = ind8[:, 128k:128k+128].T @ recb
    ind8 = np.zeros((8, 512), bf)
    for k in range(4):
        ind8[2 * k, 128 * k : 128 * k + 32] = 1.0
        ind8[2 * k + 1, 128 * k + 64 : 128 * k + 96] = 1.0
    shared = {
        "wqT": wqT,
        "wqb": wqb_p,
        "ind8": ind8,
        "fcT": fcT,
        "gbb": np.ascontiguousarray(
            np.concatenate(
                [
                    np.broadcast_to(ln_gamma, (128, 256)),
                    np.broadcast_to(ln_beta, (128, 256)),
                ],
                axis=1,
            )
        ),
    }
    in_maps = []
    for v in range(NCORES):
        m = dict(shared)
        m.update(_prep_core(x, z, fc_b, v))
        in_maps.append(m)

    res = run_bass_kernel_spmd(
        nc,
        in_maps,
        core_ids=list(range(NCORES)),
        trace=_trace,
        tmpdir=_tmpdir,
    )
    out = np.empty((BS, SEG, NV, D_IN), np.float32)
    for v in range(NCORES):
        out[:, :, v, :] = res.results[v]["y"].reshape(BS, SEG, D_IN)
    kernel._last_result = res
    return out


# revision 42
# speedup vs baseline: 1.0361x; 1.0307x over previous
"""Trainium2 Bass kernel for nn_MultiHeadEncDecAttention.

Problem (full shapes):
  x:[4,512,8,256] z:[256,512,32] w_q_w:[256,256] fc_w:[256,256] (+biases, LN params)
  q = x@w_q_w.T (+b) -> [h,v,b,s,dq]; attn = softmax(q@z^T/sqrt(dq)); out = attn@z
  o2 = concat_h(out)@fc_w.T (+b); y = LN(o2 + x)*gamma + beta

Sharding: split on n_verts (nv=8) across the 8 cores - every stage
(q-proj, attention, fc, LN) is independent per vert, so zero cross-core comms.

v2 design (per core, r = b*512+s in [0,2048)):
  Heads processed in order h_perm=[0,4,1,5,2,6,3,7]; head at order index o
  lives in tile T=o//4 at partition band 32*(o%4). Consecutive pairs hit
  different PE row-groups -> row-tiled K=32 logits matmuls overlap.
  Logits emitted per (pair, s-half 256) unit = [512t, 256s] in one 2-bank
  PSUM tile; exp on ACT per unit.
  AV: pairs (h=k, h=k+4) col-tiled at out partition bases {0,64} into one
  PSUM bank ([33] rows each + ones-row sums at rows 32/96); one DVE cast
  [97,512] lands both heads directly in outcT tile k (no shift DMAs).
  fc: 4 accumulating K=128 matmuls over the 4 outcT tiles (fcT zero-padded
  on the junk rows); norm multiply zeroes junk rows via indicator-matmul B.
"""

import sys

sys.path.insert(0, "/opt/trn_rl_repo")

from contextlib import ExitStack

import ml_dtypes
import numpy as np

import concourse.bass as bass
import concourse.tile as tile
from concourse import mybir

F32 = mybir.dt.float32
BF16 = mybir.dt.bfloat16
AX = mybir.AluOpType
AF = mybir.ActivationFunctionType

N_HEAD = 8
D_Q = 32
D_IN = 256
BS = 4
SEG = 512
NV = 8
LN_EPS = 1e-5
R = BS * SEG  # 2048 rows per core
NCORES = 8
INV_TEMP = 1.0 / np.sqrt(np.float32(D_Q))
H_PERM = [0, 4, 1, 5, 2, 6, 3, 7]  # head for order index o

_prog_cache = {}


def _build(use_wqb: bool, use_gb: bool):
    from concourse import bacc

    nc = bacc.Bacc("TRN2", target_bir_lowering=False, debug=False)

    d_xT = nc.dram_tensor("xT", [2, 128, R], BF16, kind="ExternalInput").ap()
    d_xres = nc.dram_tensor("xres", [128, 16 * 256], F32, kind="ExternalInput").ap()
    d_zT = nc.dram_tensor("zT", [2, 128, 2048], BF16, kind="ExternalInput").ap()
    d_zA = nc.dram_tensor("zA", [4, 128, 66 * 16], BF16, kind="ExternalInput").ap()
    d_wqT = nc.dram_tensor("wqT", [2, 128, 256], BF16, kind="ExternalInput").ap()
    d_wqb = nc.dram_tensor("wqb", [128, 2], F32, kind="ExternalInput").ap()
    d_fcT = nc.dram_tensor("fcT", [4, 128, D_IN], BF16, kind="ExternalInput").ap()
    d_gbb = nc.dram_tensor("gbb", [128, 512], F32, kind="ExternalInput").ap()
    d_ind8 = nc.dram_tensor("ind8", [8, 512], BF16, kind="ExternalInput").ap()
    d_y = nc.dram_tensor("y", [R, D_IN], F32, kind="ExternalOutput").ap()

    with tile.TileContext(nc) as tc, ExitStack() as ctx:
        P = ctx.enter_context  # noqa

        big = P(tc.tile_pool(name="big", bufs=1))
        lgp = P(tc.tile_pool(name="lgp", bufs=3, space="PSUM"))
        avp = P(tc.tile_pool(name="avp", bufs=2, space="PSUM"))
        expp = P(tc.tile_pool(name="expp", bufs=6))
        smp = P(tc.tile_pool(name="smp", bufs=2))
        stp = P(tc.tile_pool(name="stp", bufs=2))
        outp = P(tc.tile_pool(name="outp", bufs=3))

        # ---- persistent SBUF tiles + input DMAs
        eps_t = big.tile([128, 1], F32)
        nc.vector.memset(eps_t[:], float(LN_EPS))
        dummy_t = big.tile([128, 1], F32)
        # early Exp so the ACT table set loads during the DMA phase
        nc.scalar.activation(dummy_t[:], eps_t[:], AF.Exp)

        # tile for the PE warm-up burst
        warm_t = big.tile([128, 512], BF16, name="warm")
        nc.gpsimd.memset(warm_t[:], 0.0)

        wqT_t = [big.tile([128, 256], BF16, name=f"wqT{k}") for k in range(2)]
        for k in range(2):
            nc.sync.dma_start(wqT_t[k][:], d_wqT[k])
        # xT as separate per-chunk tiles so qproj chunk n depends only on its
        # own DMA (dep tracking for DMA writes is whole-tile)
        xT_t = [
            [big.tile([128, 512], BF16, name=f"xT{k}_{n}") for n in range(4)]
            for k in range(2)
        ]
        for k in range(2):
            nc.gpsimd.dma_start(xT_t[k][0][:], d_xT[k, :, 0:512])
        wqb_t = big.tile([128, 2], F32)
        if use_wqb:
            nc.sync.dma_start(wqb_t[:], d_wqb)
        # remaining inputs split across the sync and gpsimd issue queues
        zT_t = [big.tile([128, 2048], BF16, name=f"zT{u}") for u in range(2)]
        nc.gpsimd.dma_start(zT_t[0][:], d_zT[0])
        for n in range(1, 4):
            for k in range(2):
                nc.sync.dma_start(xT_t[k][n][:], d_xT[k, :, 512 * n : 512 * (n + 1)])
        zA_t = [big.tile([128, 66 * 16], BF16, name=f"zA{c}") for c in range(4)]
        for c in range(4):
            nc.gpsimd.dma_start(zA_t[c][:], d_zA[c])
        nc.gpsimd.dma_start(zT_t[1][:], d_zT[1])

        ind8_t = big.tile([8, 512], BF16)
        nc.sync.dma_start(ind8_t[:], d_ind8)
        fcT_t = [big.tile([128, D_IN], BF16, name=f"fcT{e}") for e in range(4)]
        for e in range(4):
            nc.sync.dma_start(fcT_t[e][:], d_fcT[e])
        gbb_t = big.tile([128, 512], F32)
        if use_gb:
            nc.sync.dma_start(gbb_t[:], d_gbb)
        xres_t = big.tile([128, 16 * 256], F32)
        nc.gpsimd.dma_start(xres_t[:], d_xres)

        qT_t = [big.tile([128, R], BF16, name=f"qT{u}") for u in range(2)]
        # outcT tile k: head k rows 0:32, sums row 32; head k+4 rows 64:96,
        # sums row 96; rows 33:63 / 97:127 junk (zeroed by the norm multiply)
        outcT = [big.tile([128, R], BF16, name=f"outcT{e}") for e in range(4)]
        yhold = big.tile([128, 16 * 256], F32)
        mvall = big.tile([128, 32], F32)

        def mm(out, lhsT, rhs, **kw):
            nc.tensor.matmul(out, lhsT, rhs, skip_group_check=True, **kw)

        # zero the never-matmul-written rows of the two av PSUM slots once,
        # so the full-partition cast below never reads non-finite stale PSUM
        for _ in range(2):
            av0 = avp.tile([128, 512], F32, tag="avb", name="av_init")
            nc.vector.memset(av0[32:64, :], 0.0)
            nc.vector.memset(av0[96:128, :], 0.0)


        # PE warm-up burst: ~10 dummy matmuls on junk data with no input deps.
        # They run during the preamble/DMA phase and push the HAM activity
        # window past its busy threshold so real matmuls start at 2.4 GHz.
        for w in range(10):
            wp = lgp.tile([128, 512], F32, tag="lg", name="warmmm")
            mm(wp[:], warm_t[:, 0:128], warm_t[:], start=True, stop=True)

        # ---- q projection chunk: qT[tile T][:, 512n:512n+512]
        def emit_qproj(T, n):
            qp = avp.tile([128, 512], F32, tag="avb", name="qp")
            for k in range(2):
                mm(
                    qp[:],
                    wqT_t[k][:, 128 * T : 128 * (T + 1)],
                    xT_t[k][n][:],
                    start=(k == 0),
                    stop=(k == 1),
                )
            dst = qT_t[T][:, 512 * n : 512 * (n + 1)]
            if use_wqb:
                nc.vector.tensor_scalar(
                    dst, qp[:], wqb_t[:, T : T + 1], 0.0, AX.add, AX.add
                )
            else:
                nc.vector.tensor_copy(dst, qp[:])

        # ---- logits + exp for one (order-index o, batch b, s-half sh) unit
        expt_tiles = {}

        def emit_logits_exp_unit(b, o, th):
            # unit = t-half: logits[t-chunks 2th:2th+2, all 512 s] for pair o
            T, beta = o // 4, 32 * (o % 4)
            if th == 0:
                expt_tiles[(b, o)] = expp.tile([128, 2048], BF16, name="expt")
            expt = expt_tiles[(b, o)]
            lt = lgp.tile([128, 1024], F32, tag="lg", name="lt")
            for j in range(2):
                c = 2 * th + j
                mm(
                    lt[:, 512 * j : 512 * (j + 1)],
                    zT_t[T][beta : beta + 32, 512 * b + 128 * c : 512 * b + 128 * (c + 1)],
                    qT_t[T][beta : beta + 32, 512 * b : 512 * (b + 1)],
                    start=True,
                    stop=True,
                    tile_position=(beta, 0),
                )
            nc.scalar.activation(
                expt[:, 1024 * th : 1024 * (th + 1)],
                lt[:],
                AF.Exp,
                scale=float(INV_TEMP),
            )

        # ---- AV for av-pair g of batch b (heads g and g+4, col-tiled {0,64})
        # split into two emission halves so the matmuls interleave between
        # later logits units (AV of group G-2 never waits on anything)
        av_state = {}

        def emit_av_half(G, half):
            b, g = divmod(G, 4)
            q = 4 * b + g
            if half == 0:
                av_state[G] = (
                    avp.tile([128, 512], F32, tag="avb", name="av"),
                    expt_tiles.pop((b, 2 * g)),
                    expt_tiles.pop((b, 2 * g + 1)),
                )
            av, eA, eB = av_state[G]
            for c in (2 * half, 2 * half + 1):
                mm(
                    av[0:33, :],
                    zA_t[c][:, 66 * q : 66 * q + 33],
                    eA[:, 512 * c : 512 * (c + 1)],
                    start=(c == 0),
                    stop=(c == 3),
                )
                mm(
                    av[64:97, :],
                    zA_t[c][:, 66 * q + 33 : 66 * q + 66],
                    eB[:, 512 * c : 512 * (c + 1)],
                    start=(c == 0),
                    stop=(c == 3),
                )
            if half == 1:
                nc.vector.tensor_copy(outcT[g][:, 512 * b : 512 * (b + 1)], av[:])
                del av_state[G]

        # ---- epilogue pieces for batch b
        sums_tiles = {}

        def emit_sums_dma(b):
            sums_b = smp.tile([8, 512], BF16, tag="sums", name="sums_b")
            sums_tiles[b] = sums_b
            for k in range(4):
                src = outcT[k][32:97:64, 512 * b : 512 * (b + 1)]
                eng = nc.sync if k % 2 == 0 else nc.gpsimd
                eng.dma_start(sums_b[2 * k : 2 * k + 2, :], src)

        B_tiles = {}

        def emit_norm_recip(b):
            sums_b = sums_tiles.pop(b)
            sumf = smp.tile([8, 512], F32, name="sumf")
            nc.vector.tensor_copy(sumf[:], sums_b[:])
            recf = smp.tile([8, 512], F32, name="recf")
            nc.vector.reciprocal_approx_fast(recf[:], sumf[:])
            recb = smp.tile([8, 512], BF16, name="recb")
            nc.vector.tensor_copy(recb[:], recf[:])
            B_tiles[b] = recb

        def emit_norm_mult(b, k):
            recb = B_tiles[b]
            Bt = avp.tile([128, 512], F32, tag="avb", name="Bt")
            mm(Bt[:], ind8_t[:, 128 * k : 128 * (k + 1)], recb[:], start=True, stop=True)
            sl = outcT[k][:, 512 * b : 512 * (b + 1)]
            nc.vector.tensor_tensor(sl, sl, Bt[:], AX.mult)

        def emit_fc_chunk(b, sc):
            ci = 4 * b + sc
            reg = avp.tile([128, 512], F32, tag="avb", name="fcp")[:, 0:256]
            for k in range(4):
                mm(
                    reg[:],
                    outcT[k][:, 512 * b + 128 * sc : 512 * b + 128 * (sc + 1)],
                    fcT_t[k][:],
                    start=(k == 0),
                    stop=(k == 3),
                )
            ysl = yhold[:, 256 * ci : 256 * (ci + 1)]
            nc.vector.tensor_tensor(
                ysl, reg[:], xres_t[:, 256 * ci : 256 * (ci + 1)], AX.add
            )
            st6 = stp.tile([128, 6], F32, name="st6")
            nc.vector.bn_stats(st6[:], ysl)
            nc.vector.bn_aggr(mvall[:, 2 * ci : 2 * ci + 2], st6[:])

        # per-chunk LN for the tail: rstd for one 128-row chunk only, so the
        # chunk's output DMA can issue while the next chunk's fc is on the PE
        def emit_ln_chunk(b, sc, eng):
            ci = 4 * b + sc
            va = stp.tile([128, 1], F32, tag="va1", name="va1")
            nc.vector.tensor_scalar(
                va[:], mvall[:, 2 * ci + 1 : 2 * ci + 2], eps_t[:], 0.0, AX.add, AX.add
            )
            y = stp.tile([128, 1], F32, tag="ny1", name="ny1")
            nc.vector.reciprocal_approx_fast(y[:], va[:])
            for _ in range(3):
                t1 = stp.tile([128, 1], F32, tag="nt11", name="nt11")
                nc.vector.tensor_tensor(t1[:], y[:], y[:], AX.mult)
                nc.vector.tensor_tensor(t1[:], t1[:], va[:], AX.mult)
                nc.vector.tensor_scalar(t1[:], t1[:], -0.5, 1.5, AX.mult, AX.add)
                nc.vector.tensor_tensor(y[:], y[:], t1[:], AX.mult)
            ysl = yhold[:, 256 * ci : 256 * (ci + 1)]
            yo = outp.tile([128, 256], F32, name="yo")
            nc.vector.tensor_scalar(
                yo[:], ysl, mvall[:, 2 * ci : 2 * ci + 1], y[:, 0:1],
                AX.subtract, AX.mult,
            )
            eng.dma_start(d_y[128 * ci : 128 * (ci + 1), :], yo[:])

        def emit_ln_finish(b):
            # rstd = 1/sqrt(var+eps) on DVE (recip-approx seed + 2 Newton
            # rsqrt iterations) - avoids the ACT sqrt table switch entirely.
            mvb = mvall[:, 8 * b : 8 * (b + 1)].rearrange("p (c two) -> p c two", two=2)
            va = stp.tile([128, 4], F32, tag="va", name="va")
            nc.vector.tensor_scalar(va[:], mvb[:, :, 1:2], eps_t[:], 0.0, AX.add, AX.add)
            y = stp.tile([128, 4], F32, tag="ny", name="ny")
            nc.vector.reciprocal_approx_fast(y[:], va[:])
            for _ in range(3):
                t1 = stp.tile([128, 4], F32, tag="nt1", name="nt1")
                nc.vector.tensor_tensor(t1[:], y[:], y[:], AX.mult)
                nc.vector.tensor_tensor(t1[:], t1[:], va[:], AX.mult)
                nc.vector.tensor_scalar(t1[:], t1[:], -0.5, 1.5, AX.mult, AX.add)
                nc.vector.tensor_tensor(y[:], y[:], t1[:], AX.mult)
            for sc in range(4):
                ci = 4 * b + sc
                ysl = yhold[:, 256 * ci : 256 * (ci + 1)]
                yo = outp.tile([128, 256], F32, name="yo")
                if use_gb:
                    t2 = outp.tile([128, 256], F32, tag="t1", name="t2")
                    nc.vector.scalar_tensor_tensor(
                        t2[:], ysl, mvall[:, 2 * ci : 2 * ci + 1], gbb_t[:, 0:256],
                        AX.subtract, AX.mult,
                    )
                    nc.vector.scalar_tensor_tensor(
                        yo[:], t2[:], y[:, sc : sc + 1], gbb_t[:, 256:512],
                        AX.mult, AX.add,
                    )
                else:
                    nc.vector.tensor_scalar(
                        yo[:], ysl, mvall[:, 2 * ci : 2 * ci + 1], y[:, sc : sc + 1],
                        AX.subtract, AX.mult,
                    )
                eng = nc.gpsimd if sc % 2 == 0 else nc.sync
                eng.dma_start(d_y[128 * ci : 128 * (ci + 1), :], yo[:])

        # ---- main schedule -------------------------------------------------
        # qproj for batch 0 first (only needs wqT + xT chunk 0)
        emit_qproj(0, 0)
        emit_qproj(1, 0)
        # bridge dummies: keep the PE busy between qproj and the first logits
        # (which wait on the zT DMA) so the HAM window never sees an idle
        # stretch before steady state - this pins the warm-clock equilibrium
        for w in range(8):
            wp = lgp.tile([128, 256], F32, tag="lg", name="bridge")
            mm(wp[:], warm_t[:, 0:128], warm_t[:, 0:256], start=True, stop=True)

        def emit_av(b, g):
            emit_av_half(4 * b + g, 0)
            emit_av_half(4 * b + g, 1)

        # epilogue pieces of batch b-1, spread across batch b's groups
        def emit_epilogue_piece(b, g):
            if b < 0:
                return
            if g == 0:
                # DVE-only: the recip chain runs while PE streams the next
                # group, so the B matmuls at g=1 never stall the PE queue
                emit_norm_recip(b)
            elif g == 1:
                for k in range(4):
                    emit_norm_mult(b, k)
                emit_fc_chunk(b, 0)
            elif g == 2:
                emit_fc_chunk(b, 1)
                emit_fc_chunk(b, 2)
            else:
                emit_fc_chunk(b, 3)
                emit_ln_finish(b)

        for b in range(BS):
            for g in range(4):
                emit_logits_exp_unit(b, 2 * g, 0)
                emit_logits_exp_unit(b, 2 * g + 1, 0)
                if b < BS - 1 and g == 1:
                    emit_qproj(0, b + 1)
                emit_logits_exp_unit(b, 2 * g, 1)
                emit_logits_exp_unit(b, 2 * g + 1, 1)
                if b < BS - 1 and g == 2:
                    emit_qproj(1, b + 1)
                # AV of the previous group (pipelined one group back)
                if g > 0:
                    emit_av(b, g - 1)
                elif b > 0:
                    emit_av(b - 1, 3)
                    emit_sums_dma(b - 1)
                emit_epilogue_piece(b - 1, g)
        emit_av(BS - 1, 3)
        emit_sums_dma(BS - 1)
        for g in range(4):
            emit_epilogue_piece(BS - 1, g)

    nc.compile()
    return nc


def _prep_core(x, z, fc_b, v):
    """Build the per-core input map (host-side layout packing) for vert v."""
    bf = ml_dtypes.bfloat16
    xv = np.ascontiguousarray(x[:, :, v, :]).reshape(R, D_IN)  # [r, d]
    xT = np.ascontiguousarray(xv.T).astype(bf).reshape(2, 128, R)  # [d, r]
    xres = np.ascontiguousarray(
        (xv + fc_b[None, :]).reshape(16, 128, 256).transpose(1, 0, 2).reshape(128, 16 * 256)
    )
    zv = z.reshape(N_HEAD, NV, BS, SEG, D_Q)[:, v]  # [h, b, t, d]
    zTp = zv.transpose(0, 1, 3, 2)  # [h, b, d, t]
    zT = np.zeros((2, 4, 32, 4, 512), bf)
    for o, h in enumerate(H_PERM):
        for b in range(BS):
            zT[o // 4, o % 4, :, b] = zTp[h, b]
    zT = np.ascontiguousarray(zT.reshape(2, 128, 2048))
    # zA: per av-pair slot q = 4b+g: [headA(g) 33 | headB(g+4) 33]
    zA = np.zeros((4, 128, 66 * 16), bf)
    za_full = np.concatenate(
        [zv, np.ones((N_HEAD, BS, SEG, 1), np.float32)], axis=-1
    ).astype(bf)  # [h, b, t, 33]
    for b in range(BS):
        for g in range(4):
            q = 4 * b + g
            for c in range(4):
                zA[c, :, 66 * q : 66 * q + 33] = za_full[g, b, 128 * c : 128 * (c + 1), :]
                zA[c, :, 66 * q + 33 : 66 * q + 66] = za_full[
                    g + 4, b, 128 * c : 128 * (c + 1), :
                ]
    return {"xT": xT, "xres": xres, "zT": zT, "zA": zA}


def kernel(x, z, w_q_w, w_q_b, fc_w, fc_b, ln_gamma, ln_beta, _trace=False, _tmpdir=None):
    from concourse.bass_utils import run_bass_kernel_spmd

    x = np.asarray(x, np.float32)
    z = np.asarray(z, np.float32)
    w_q_w = np.asarray(w_q_w, np.float32)
    w_q_b = np.asarray(w_q_b, np.float32)
    fc_w = np.asarray(fc_w, np.float32)
    fc_b = np.asarray(fc_b, np.float32)
    ln_gamma = np.asarray(ln_gamma, np.float32)
    ln_beta = np.asarray(ln_beta, np.float32)

    use_wqb = bool(np.any(w_q_b != 0.0))
    use_gb = bool(np.any(ln_gamma != 1.0) or np.any(ln_beta != 0.0))

    key = (use_wqb, use_gb)
    if key not in _prog_cache:
        _prog_cache[key] = _build(use_wqb, use_gb)
    nc = _prog_cache[key]

    bf = ml_dtypes.bfloat16
    # e' permutation: tile T col j -> head H_PERM[4T + j//32], dq j%32
    eperm = np.zeros(256, np.int64)
    for T in range(2):
        for j in range(128):
            o = 4 * T + j // 32
            eperm[128 * T + j] = 32 * H_PERM[o] + j % 32
    wqT = np.ascontiguousarray(w_q_w.T[:, eperm]).astype(bf).reshape(2, 128, 256)
    wqb_p = np.zeros((128, 2), np.float32)
    for T in range(2):
        wqb_p[:, T] = w_q_b[eperm[128 * T : 128 * (T + 1)]]
    # fcT tile k: rows 0:32 = fc rows of head k, rows 64:96 = head k+4, else 0
    fcT_full = fc_w.T  # [e, d_in]
    fcT = np.zeros((4, 128, D_IN), np.float32)
    for k in range(4):
        fcT[k, 0:32] = fcT_full[32 * k : 32 * (k + 1)]
        fcT[k, 64:96] = fcT_full[32 * (k + 4) : 32 * (k + 5)]
    fcT = fcT.astype(bf)
    # ind8 tile: B_k = ind8[:, 128k:128k+128].T @ recb
    ind8 = np.zeros((8, 512), bf)
    for k in range(4):
        ind8[2 * k, 128 * k : 128 * k + 32] = 1.0
        ind8[2 * k + 1, 128 * k + 64 : 128 * k + 96] = 1.0
    shared = {
        "wqT": wqT,
        "wqb": wqb_p,
        "ind8": ind8,
        "fcT": fcT,
        "gbb": np.ascontiguousarray(
            np.concatenate(
                [
                    np.broadcast_to(ln_gamma, (128, 256)),
                    np.broadcast_to(ln_beta, (128, 256)),
                ],
                axis=1,
            )
        ),
    }
    in_maps = []
    for v in range(NCORES):
        m = dict(shared)
        m.update(_prep_core(x, z, fc_b, v))
        in_maps.append(m)

    res = run_bass_kernel_spmd(
        nc,
        in_maps,
        core_ids=list(range(NCORES)),
        trace=_trace,
        tmpdir=_tmpdir,
    )
    out = np.empty((BS, SEG, NV, D_IN), np.float32)
    for v in range(NCORES):
        out[:, :, v, :] = res.results[v]["y"].reshape(BS, SEG, D_IN)
    kernel._last_result = res
    return out
